# revision 47
# baseline (speedup 1.0000x reference)
"""Trainium2 Bass kernel for the hybrid attention head (nn_AttentionHead_Hybrid).

Math (per batch):
    norms  n_i = ||x_i||;  xh = x / n
    O      = product of 2016 Givens rotations (built on host, fp32)
    S[i,j] = xh_i . O . xh_j
    A      = S^2 * n_i n_j ;  P = softmax(A / 8)
    V      = x @ Vw^T + Vb
    out    = LayerNorm(P @ V + x) * gamma + beta

Device formulation (per core, 4 batches):
    W'     = diag(s') X with s'_n = ||x_n||^-1/2 * 8^-1/4 * (128/ln2)^1/4
    gt     = (W' O)^T  (host-prepped, f16)          so R[j,i] = w'_j O^T w'_i
    R^2    = (128/ln2) * A/8
    E      = exp(A/8) via Schraudolph: bitcast_bf16(int16(R^2 + B))  [one DVE
             tensor_scalar in 4x mode; the sawtooth error cancels in softmax]
    Vt     = [X Vw^T | 1]  (bf16; ones column gives softmax row-sums for free;
             V_b folded into the residual on host: xr = x + V_b)
    OUT^T  = sum_j Vt[j,:]^T E^T[j,:]   in [65, N] psum -> DMA to SBUF ->
             PE-transpose back -> y = OUT*rcol + xr -> LayerNorm (bn_stats)
    out    bf16, converted to f32 on host.

Sharding: data-parallel over batch, 4 batches per core on 8 cores.
"""

import math

import numpy as np
import ml_dtypes

import concourse.bacc as bacc
import concourse.bass as bass
import concourse.tile as tile
from concourse import bass_utils, mybir

AF = mybir.ActivationFunctionType
ALU = mybir.AluOpType
DT = mybir.dt

B, N, D = 32, 1024, 64
NCORES = 8
BPC = B // NCORES          # batches per core
NT = N // 128              # 128-row tiles per batch

SCHRAUD_C4 = 128.0 / math.log(2.0)          # scale absorbed into W' (4th root)
SCHRAUD_B = 16256.0 - 128.0 * 0.0430        # bf16 exponent bias, sigma-centered

# of the 8 per-batch [128,1024] squares, how many run on ACT (rest on DVE)
K_ACT_SQ = 8


def _build_orthogonal(phi: np.ndarray, d: int = D) -> np.ndarray:
    """fp32 replica of the reference jax.lax.scan Givens chain."""
    O = np.eye(d, dtype=np.float32)
    ii, jj = np.triu_indices(d, k=1)
    c = np.cos(phi.astype(np.float32))
    s = np.sin(phi.astype(np.float32))
    for k in range(len(phi)):
        i, j = int(ii[k]), int(jj[k])
        ri = O[i].copy()
        rj = O[j].copy()
        O[i] = c[k] * ri + s[k] * rj
        O[j] = -s[k] * ri + c[k] * rj
    return O


def _build_nc(apply_gamma_beta: bool):
    nc = bacc.Bacc("TRN2", target_bir_lowering=False)

    wt_t = nc.dram_tensor("wt", [BPC, D, N], DT.float16, kind="ExternalInput")
    gt_t = nc.dram_tensor("gt", [BPC, D, N], DT.float16, kind="ExternalInput")
    v_t = nc.dram_tensor("v", [BPC, N, 66], DT.bfloat16, kind="ExternalInput")
    xr_t = nc.dram_tensor("xr", [BPC, N, D], DT.bfloat16, kind="ExternalInput")
    xs_t = nc.dram_tensor("xs", [BPC, N], DT.float32, kind="ExternalInput")
    id_t = nc.dram_tensor("ident", [128, 128], DT.float32, kind="ExternalInput")
    gb_t = nc.dram_tensor("gb", [2, D], DT.float32, kind="ExternalInput")
    out_t = nc.dram_tensor("out", [BPC, N, D], DT.bfloat16, kind="ExternalOutput")

    with tile.TileContext(nc) as tc:
        with (
            tc.tile_pool(name="const", bufs=1) as constp,
            tc.tile_pool(name="loadp", bufs=2) as loadp,
            tc.tile_pool(name="asqp", bufs=4) as asqp,
            tc.tile_pool(name="ep", bufs=4) as ep,
            tc.tile_pool(name="vp", bufs=2) as vp,
            tc.tile_pool(name="otp", bufs=3) as otp,
            tc.tile_pool(name="yp", bufs=2) as yp,
            tc.tile_pool(name="outp", bufs=2) as outp,
            tc.tile_pool(name="statp", bufs=3) as statp,
            tc.tile_pool(name="psA", bufs=3, space="PSUM") as psA,
            tc.tile_pool(name="psB", bufs=1, space="PSUM") as psB,
        ):
            def emit_consts():
                id_sb = constp.tile([128, 128], DT.float32)
                nc.sync.dma_start(out=id_sb, in_=id_t[:, :])
                idb_sb = constp.tile([128, 128], DT.bfloat16)
                nc.vector.tensor_copy(idb_sb, id_sb)
                gam_sb = bet_sb = None
                if apply_gamma_beta:
                    gam_sb = constp.tile([128, D], DT.float32)
                    nc.sync.dma_start(
                        out=gam_sb, in_=gb_t[0, :].to_broadcast([128, D])
                    )
                    bet_sb = constp.tile([128, D], DT.float32)
                    nc.sync.dma_start(
                        out=bet_sb, in_=gb_t[1, :].to_broadcast([128, D])
                    )
                return id_sb, idb_sb, gam_sb, bet_sb

            def emit_loads(b):
                wt = loadp.tile([128, N], DT.float16, tag="wt")
                nc.sync.dma_start(out=wt[0:D, :], in_=wt_t[b])
                nc.sync.dma_start(out=wt[D : 2 * D, :], in_=wt_t[b])
                gt = loadp.tile([128, N], DT.float16, tag="gt")
                nc.sync.dma_start(out=gt[0:D, :], in_=gt_t[b])
                nc.sync.dma_start(out=gt[D : 2 * D, :], in_=gt_t[b])
                v_sb = loadp.tile([128, NT, 66], DT.bfloat16, tag="v")
                nc.sync.dma_start(
                    out=v_sb, in_=v_t[b].rearrange("(t p) c -> p t c", p=128)
                )
                xr = loadp.tile([128, NT, D], DT.bfloat16, tag="xr")
                nc.sync.dma_start(
                    out=xr, in_=xr_t[b].rearrange("(t p) d -> p t d", p=128)
                )
                xs = loadp.tile([128, NT], DT.float32, tag="xs")
                nc.sync.dma_start(
                    out=xs, in_=xs_t[b].rearrange("(t p) -> p t", p=128)
                )
                return wt, gt, v_sb, xr, xs

            # batch-0 wt/gt loads lead the DMA queues (first S depends on
            # them); V comes precomputed from the host.
            state = {0: emit_loads(0)}
            id_sb, idb_sb, gam_sb, bet_sb = emit_consts()

            def emit_epilogue_grp(b, ot, xr, xs, st, grp):
                # ---- transpose back; y = OUT*(1/rowsum) + xr ----
                # ptr cols: 0..63 num, 64 sum_d num, 65 denominator
                (rcol, mus, y, ys2, mu, m2, ve, sd, rstd, mub, rstdb,
                 out_sb) = st
                out_dram = out_t[b].rearrange("(t p) d -> p t d", p=128)
                if True:
                    g_sl = slice(grp * 4, grp * 4 + 4)
                    ptr = psA.tile([128, 4, D + 2], DT.bfloat16, tag="pr", name="ptr")
                    for qq in range(4):
                        it = grp * 4 + qq
                        nc.tensor.transpose(
                            ptr[:, qq, :],
                            ot[0 : D + 2, it * 128 : (it + 1) * 128],
                            idb_sb[0 : D + 2, 0 : D + 2],
                        )
                    den = statp.tile([128, 4], DT.float32, tag="den")
                    nc.vector.tensor_copy(den, ptr[:, :, D + 1])
                    nc.vector.reciprocal_approx_fast(rcol[:, g_sl], den)
                    rc4 = rcol[:, g_sl]
                    rcol_bc = bass.AP(
                        tensor=rcol.tensor, offset=rc4.offset,
                        ap=[rcol.ap[0], [1, 4], [0, D]],
                    )
                    nc.vector.tensor_tensor(
                        out=y[:, g_sl, :], in0=ptr[:, :, 0:D], in1=rcol_bc,
                        op=ALU.mult,
                    )
                    nc.vector.tensor_tensor(
                        out=mus[:, g_sl], in0=ptr[:, :, D], in1=rc4, op=ALU.mult
                    )
                    nc.vector.tensor_add(
                        y[:, g_sl, :], y[:, g_sl, :], xr[:, g_sl, :]
                    )

                    # LN stats: mean from the free sum_d num column, variance
                    # from one squared reduce
                    ysq = yp.tile([128, 4, D], DT.bfloat16, tag="ysq")
                    nc.vector.tensor_mul(ysq, y[:, g_sl, :], y[:, g_sl, :])
                    nc.vector.reduce_sum(
                        ys2[:, g_sl], ysq, axis=mybir.AxisListType.X
                    )
                    # grp0 stats on Pool, grp1 on DVE: the two groups' serial
                    # LN chains then run in parallel (matters for the tail)
                    st_eng = nc.gpsimd if grp == 0 else nc.vector
                    st_eng.tensor_scalar(
                        mu[:, g_sl], mus[:, g_sl],
                        scalar1=1.0 / D, scalar2=None, op0=ALU.mult,
                    )
                    st_eng.tensor_add(mu[:, g_sl], mu[:, g_sl], xs[:, g_sl])
                    # var + eps = ys2/64 - mu^2 + 1e-5
                    st_eng.tensor_mul(m2[:, g_sl], mu[:, g_sl], mu[:, g_sl])
                    st_eng.tensor_scalar(
                        ve[:, g_sl], ys2[:, g_sl],
                        scalar1=1.0 / D, scalar2=1e-5, op0=ALU.mult, op1=ALU.add,
                    )
                    st_eng.tensor_sub(ve[:, g_sl], ve[:, g_sl], m2[:, g_sl])
                    nc.scalar.activation(sd[:, g_sl], ve[:, g_sl], AF.Sqrt)
                    nc.vector.reciprocal_approx_fast(rstd[:, g_sl], sd[:, g_sl])
                    nc.vector.tensor_copy(mub[:, g_sl], mu[:, g_sl])
                    nc.vector.tensor_copy(rstdb[:, g_sl], rstd[:, g_sl])

                    # normalize: out = (y - mu) * rstd
                    mub_bc = bass.AP(
                        tensor=mub.tensor, offset=mub[:, g_sl].offset,
                        ap=[mub.ap[0], [1, 4], [0, D]],
                    )
                    rstdb_bc = bass.AP(
                        tensor=rstdb.tensor, offset=rstdb[:, g_sl].offset,
                        ap=[rstdb.ap[0], [1, 4], [0, D]],
                    )
                    nc.vector.tensor_tensor(
                        out=out_sb[:, g_sl, :], in0=y[:, g_sl, :], in1=mub_bc,
                        op=ALU.subtract,
                    )
                    nc.vector.tensor_tensor(
                        out=out_sb[:, g_sl, :], in0=out_sb[:, g_sl, :],
                        in1=rstdb_bc, op=ALU.mult,
                    )
                    if apply_gamma_beta:
                        for t in range(grp * 4, grp * 4 + 4):
                            nc.gpsimd.tensor_mul(
                                out_sb[:, t, :], out_sb[:, t, :], gam_sb
                            )
                            nc.gpsimd.tensor_add(
                                out_sb[:, t, :], out_sb[:, t, :], bet_sb
                            )
                    nc.sync.dma_start(
                        out=out_dram[:, g_sl, :], in_=out_sb[:, g_sl, :]
                    )

            def make_stats():
                rcol = statp.tile([128, NT], DT.float32, tag="rcol")
                mus = statp.tile([128, NT], DT.float32, tag="mus")
                y = yp.tile([128, NT, D], DT.bfloat16, tag="y")
                ys2 = statp.tile([128, NT], DT.float32, tag="ys2")
                mu = statp.tile([128, NT], DT.float32, tag="mu")
                m2 = statp.tile([128, NT], DT.float32, tag="m2")
                ve = statp.tile([128, NT], DT.float32, tag="ve")
                sd = statp.tile([128, NT], DT.float32, tag="sd")
                rstd = statp.tile([128, NT], DT.float32, tag="rstd")
                mub = statp.tile([128, NT], DT.bfloat16, tag="mub")
                rstdb = statp.tile([128, NT], DT.bfloat16, tag="rstdb")
                out_sb = outp.tile([128, NT, D], DT.bfloat16, tag="o")
                return (rcol, mus, y, ys2, mu, m2, ve, sd, rstd, mub, rstdb,
                        out_sb)

            pending = None
            carry = None
            for b in range(BPC):
                wt, gt, v_sb, xr, xs = state[b]

                pot = psB.tile([128, N], DT.float32, tag="pot", name="pot")
                ebs = {}

                # ---- per j-tile: R via both PE row groups on disjoint column
                # halves of the same tile (adjacent emission -> they overlap),
                # then square, schraudolph-exp, PV accumulate (lagged one tile
                # so the PV matmuls never wait on the exp) ----
                for jt in range(NT):
                    pr = psA.tile([128, N], DT.float32, tag="pr", name="pr")
                    nc.tensor.matmul(
                        pr[:, 0:512],
                        lhsT=wt[0:D, jt * 128 : (jt + 1) * 128],
                        rhs=gt[0:D, 0:512],
                        tile_position=(0, 0),
                    )
                    nc.tensor.matmul(
                        pr[:, 512:N],
                        lhsT=wt[D : 2 * D, jt * 128 : (jt + 1) * 128],
                        rhs=gt[D : 2 * D, 512:N],
                        tile_position=(64, 0),
                    )
                    asq = asqp.tile([128, N], DT.float16, tag="asq")
                    if jt < K_ACT_SQ:
                        nc.scalar.activation(asq, pr, AF.Square)
                    else:
                        rf = asqp.tile([128, N], DT.float16, tag="rf")
                        nc.vector.tensor_copy(rf, pr)
                        nc.vector.tensor_mul(asq, rf, rf)
                    e = ep.tile([128, N], DT.uint16, tag="e")
                    nc.vector.tensor_scalar(
                        e, asq, scalar1=SCHRAUD_B, scalar2=None, op0=ALU.add
                    )
                    ebs[jt] = e.bitcast(DT.bfloat16)
                    if jt == 0 and carry is not None:
                        # previous batch's final PV, emitted after this
                        # batch's first S so the square pipeline restarts
                        # without waiting behind the PV bunch
                        cv, cebs, cpot = carry
                        for c in range(2):
                            nc.tensor.matmul(
                                cpot[0 : D + 2, c * 512 : (c + 1) * 512],
                                lhsT=cv[:, NT - 1, 0 : D + 2],
                                rhs=cebs[NT - 1][:, c * 512 : (c + 1) * 512],
                                start=False,
                                stop=True,
                            )
                        carry = None
                    if jt >= 1:
                        pj = jt - 1
                        for c in range(2):
                            nc.tensor.matmul(
                                pot[0 : D + 2, c * 512 : (c + 1) * 512],
                                lhsT=v_sb[:, pj, 0 : D + 2],
                                rhs=ebs[pj][:, c * 512 : (c + 1) * 512],
                                start=(pj == 0),
                                stop=False,
                            )
                    if jt == 1 and pending is not None:
                        # previous batch's OUT^T -> SBUF copy (half DVE, half
                        # ACT), deferred so this batch's squares lead the ACT
                        # queue
                        pb, ppot, pxr, pxs = pending
                        ot = otp.tile([128, N], DT.bfloat16, tag="ot")
                        nc.vector.tensor_copy(
                            ot[0 : D + 2, 0:768], ppot[0 : D + 2, 0:768]
                        )
                        nc.scalar.copy(
                            ot[0 : D + 2, 768:N], ppot[0 : D + 2, 768:N]
                        )
                        pending = (pb, ot, pxr, pxs)
                    if jt == 2 and b + 1 < BPC:
                        state[b + 1] = emit_loads(b + 1)
                    if jt == 3 and pending is not None:
                        # previous batch's transposes + LN, deferred so the PE
                        # has this batch's matmuls in flight while the OT copy
                        # completes; split per group to spread the PE burst
                        pending_st = make_stats()
                        emit_epilogue_grp(*pending, pending_st, 0)
                    if jt == 5 and pending is not None:
                        emit_epilogue_grp(*pending, pending_st, 1)
                        pending = None

                pending = (b, pot, xr, xs)
                carry = (v_sb, ebs, pot)

            cv, cebs, cpot = carry
            for c in range(2):
                nc.tensor.matmul(
                    cpot[0 : D + 2, c * 512 : (c + 1) * 512],
                    lhsT=cv[:, NT - 1, 0 : D + 2],
                    rhs=cebs[NT - 1][:, c * 512 : (c + 1) * 512],
                    start=False,
                    stop=True,
                )

            pb, ppot, pxr, pxs = pending
            ot = otp.tile([128, N], DT.bfloat16, tag="ot")
            nc.vector.tensor_copy(ot[0 : D + 2, 0:512], ppot[0 : D + 2, 0:512])
            nc.scalar.copy(ot[0 : D + 2, 512:N], ppot[0 : D + 2, 512:N])
            st = make_stats()
            emit_epilogue_grp(pb, ot, pxr, pxs, st, 0)
            emit_epilogue_grp(pb, ot, pxr, pxs, st, 1)

    nc.compile()
    return nc


_NC_CACHE: dict = {}


def kernel(input1, V_w, V_b, phi, ln_gamma, ln_beta, _trace=False):
    input1 = np.ascontiguousarray(np.asarray(input1, dtype=np.float32))
    V_w = np.asarray(V_w, dtype=np.float32)
    V_b = np.asarray(V_b, dtype=np.float32)
    phi = np.asarray(phi, dtype=np.float32)
    ln_gamma = np.asarray(ln_gamma, dtype=np.float32)
    ln_beta = np.asarray(ln_beta, dtype=np.float32)

    apply_gb = not (np.all(ln_gamma == 1.0) and np.all(ln_beta == 0.0))

    if apply_gb not in _NC_CACHE:
        _NC_CACHE[apply_gb] = _build_nc(apply_gb)
    nc = _NC_CACHE[apply_gb]

    O = _build_orthogonal(phi)
    ident = np.eye(128, dtype=np.float32)
    gb = np.ascontiguousarray(np.stack([ln_gamma, ln_beta]).astype(np.float32))

    # host prep: W' = x * ||x||^-1/2 * 8^-1/4 * (128/ln2)^1/4 (transposed f16),
    # gt = (W' O)^T, xo = X^T, xr = x + V_b (bf16 residual w/ folded bias)
    ss = (input1.astype(np.float64) ** 2).sum(-1)
    s = (ss ** -0.25 * 8.0 ** -0.25 * SCHRAUD_C4 ** 0.25).astype(np.float32)
    w = input1 * s[..., None]
    wt_full = np.ascontiguousarray(w.transpose(0, 2, 1).astype(np.float16))
    gt_full = np.ascontiguousarray(
        (w @ O).transpose(0, 2, 1).astype(np.float16)
    )
    vfull = input1 @ V_w.T
    v_full = np.ascontiguousarray(
        np.concatenate(
            [
                vfull,
                vfull.sum(-1, keepdims=True),
                np.ones((B, N, 1), np.float32),
            ],
            axis=-1,
        ).astype(ml_dtypes.bfloat16)
    )
    xr_f32 = input1 + V_b
    xr_full = np.ascontiguousarray(xr_f32.astype(ml_dtypes.bfloat16))
    xs_full = np.ascontiguousarray((xr_f32.sum(-1) / D).astype(np.float32))

    in_maps = []
    for c in range(NCORES):
        sl = slice(c * BPC, (c + 1) * BPC)
        in_maps.append(
            {
                "wt": wt_full[sl],
                "gt": gt_full[sl],
                "v": v_full[sl],
                "xr": xr_full[sl],
                "xs": xs_full[sl],
                "ident": ident,
                "gb": gb,
            }
        )

    res = bass_utils.run_bass_kernel_spmd(
        nc, in_maps, core_ids=list(range(NCORES)), trace=_trace
    )
    out = np.concatenate(
        [res.results[c]["out"].astype(np.float32) for c in range(NCORES)], axis=0
    )
    if _trace:
        kernel._last_result = res
    return out


# revision 48
# speedup vs baseline: 1.0396x; 1.0396x over previous
"""Trainium2 Bass kernel for the hybrid attention head (nn_AttentionHead_Hybrid).

Math (per batch):
    norms  n_i = ||x_i||;  xh = x / n
    O      = product of 2016 Givens rotations (built on host, fp32)
    S[i,j] = xh_i . O . xh_j
    A      = S^2 * n_i n_j ;  P = softmax(A / 8)
    V      = x @ Vw^T + Vb
    out    = LayerNorm(P @ V + x) * gamma + beta

Device formulation (per core, 4 batches):
    W'     = diag(s') X with s'_n = ||x_n||^-1/2 * 8^-1/4 * (128/ln2)^1/4
    gt     = (W' O)^T  (host-prepped, f16)          so R[j,i] = w'_j O^T w'_i
    R^2    = (128/ln2) * A/8
    E      = exp(A/8) via Schraudolph: bitcast_bf16(int16(R^2 + B))  [one DVE
             tensor_scalar in 4x mode; the sawtooth error cancels in softmax]
    Vt     = [X Vw^T | 1]  (bf16; ones column gives softmax row-sums for free;
             V_b folded into the residual on host: xr = x + V_b)
    OUT^T  = sum_j Vt[j,:]^T E^T[j,:]   in [65, N] psum -> DMA to SBUF ->
             PE-transpose back -> y = OUT*rcol + xr -> LayerNorm (bn_stats)
    out    bf16, converted to f32 on host.

Sharding: data-parallel over batch, 4 batches per core on 8 cores.
"""

import math

import numpy as np
import ml_dtypes

import concourse.bacc as bacc
import concourse.bass as bass
import concourse.tile as tile
from concourse import bass_utils, mybir

AF = mybir.ActivationFunctionType
ALU = mybir.AluOpType
DT = mybir.dt

B, N, D = 32, 1024, 64
NCORES = 8
BPC = B // NCORES          # batches per core
NT = N // 128              # 128-row tiles per batch

SCHRAUD_C4 = 128.0 / math.log(2.0)          # scale absorbed into W' (4th root)
SCHRAUD_B = 16256.0 - 128.0 * 0.0430        # bf16 exponent bias, sigma-centered

# of the 8 per-batch [128,1024] squares, how many run on ACT (rest on DVE)
K_ACT_SQ = 8


def _build_orthogonal(phi: np.ndarray, d: int = D) -> np.ndarray:
    """fp32 replica of the reference jax.lax.scan Givens chain."""
    O = np.eye(d, dtype=np.float32)
    ii, jj = np.triu_indices(d, k=1)
    c = np.cos(phi.astype(np.float32))
    s = np.sin(phi.astype(np.float32))
    for k in range(len(phi)):
        i, j = int(ii[k]), int(jj[k])
        ri = O[i].copy()
        rj = O[j].copy()
        O[i] = c[k] * ri + s[k] * rj
        O[j] = -s[k] * ri + c[k] * rj
    return O


def _build_nc(apply_gamma_beta: bool):
    nc = bacc.Bacc("TRN2", target_bir_lowering=False)

    wt_t = nc.dram_tensor("wt", [BPC, D, N], DT.float16, kind="ExternalInput")
    gt_t = nc.dram_tensor("gt", [BPC, D, N], DT.float16, kind="ExternalInput")
    v_t = nc.dram_tensor("v", [BPC, N, 66], DT.bfloat16, kind="ExternalInput")
    xr_t = nc.dram_tensor("xr", [BPC, N, D], DT.bfloat16, kind="ExternalInput")
    xs_t = nc.dram_tensor("xs", [BPC, N], DT.float32, kind="ExternalInput")
    id_t = nc.dram_tensor("ident", [128, 128], DT.float32, kind="ExternalInput")
    gb_t = nc.dram_tensor("gb", [2, D], DT.float32, kind="ExternalInput")
    out_t = nc.dram_tensor("out", [BPC, N, D], DT.bfloat16, kind="ExternalOutput")

    with tile.TileContext(nc) as tc:
        with (
            tc.tile_pool(name="const", bufs=1) as constp,
            tc.tile_pool(name="loadp", bufs=2) as loadp,
            tc.tile_pool(name="asqp", bufs=4) as asqp,
            tc.tile_pool(name="ep", bufs=4) as ep,
            tc.tile_pool(name="vp", bufs=2) as vp,
            tc.tile_pool(name="otp", bufs=3) as otp,
            tc.tile_pool(name="yp", bufs=2) as yp,
            tc.tile_pool(name="outp", bufs=2) as outp,
            tc.tile_pool(name="statp", bufs=3) as statp,
            tc.tile_pool(name="psA", bufs=3, space="PSUM") as psA,
            tc.tile_pool(name="psB", bufs=1, space="PSUM") as psB,
        ):
            def emit_consts():
                id_sb = constp.tile([128, 128], DT.float32)
                nc.sync.dma_start(out=id_sb, in_=id_t[:, :])
                idb_sb = constp.tile([128, 128], DT.bfloat16)
                nc.vector.tensor_copy(idb_sb, id_sb)
                gam_sb = bet_sb = None
                if apply_gamma_beta:
                    gam_sb = constp.tile([128, D], DT.float32)
                    nc.sync.dma_start(
                        out=gam_sb, in_=gb_t[0, :].to_broadcast([128, D])
                    )
                    bet_sb = constp.tile([128, D], DT.float32)
                    nc.sync.dma_start(
                        out=bet_sb, in_=gb_t[1, :].to_broadcast([128, D])
                    )
                return id_sb, idb_sb, gam_sb, bet_sb

            def emit_loads(b):
                wt = loadp.tile([128, N], DT.float16, tag="wt")
                nc.sync.dma_start(out=wt[0:D, :], in_=wt_t[b])
                nc.sync.dma_start(out=wt[D : 2 * D, :], in_=wt_t[b])
                gt = loadp.tile([128, N], DT.float16, tag="gt")
                nc.sync.dma_start(out=gt[0:D, :], in_=gt_t[b])
                nc.sync.dma_start(out=gt[D : 2 * D, :], in_=gt_t[b])
                v_sb = loadp.tile([128, NT, 66], DT.bfloat16, tag="v")
                nc.sync.dma_start(
                    out=v_sb, in_=v_t[b].rearrange("(t p) c -> p t c", p=128)
                )
                xr = loadp.tile([128, NT, D], DT.bfloat16, tag="xr")
                nc.sync.dma_start(
                    out=xr, in_=xr_t[b].rearrange("(t p) d -> p t d", p=128)
                )
                xs = loadp.tile([128, NT], DT.float32, tag="xs")
                nc.sync.dma_start(
                    out=xs, in_=xs_t[b].rearrange("(t p) -> p t", p=128)
                )
                return wt, gt, v_sb, xr, xs

            # batch-0 wt/gt loads lead the DMA queues (first S depends on
            # them); V comes precomputed from the host.
            state = {0: emit_loads(0)}
            id_sb, idb_sb, gam_sb, bet_sb = emit_consts()

            def emit_epilogue_grp(b, ot, xr, xs, st, grp):
                # ---- transpose back; y = OUT*(1/rowsum) + xr ----
                # ptr cols: 0..63 num, 64 sum_d num, 65 denominator
                (rcol, mus, y, ys2, mu, m2, ve, sd, rstd, mub, rstdb,
                 out_sb) = st
                out_dram = out_t[b].rearrange("(t p) d -> p t d", p=128)
                if True:
                    g_sl = slice(grp * 4, grp * 4 + 4)
                    ptr = psA.tile([128, 4, D + 2], DT.bfloat16, tag="pr", name="ptr")
                    for qq in range(4):
                        it = grp * 4 + qq
                        nc.tensor.transpose(
                            ptr[:, qq, :],
                            ot[0 : D + 2, it * 128 : (it + 1) * 128],
                            idb_sb[0 : D + 2, 0 : D + 2],
                        )
                    den = statp.tile([128, 4], DT.float32, tag="den")
                    nc.vector.tensor_copy(den, ptr[:, :, D + 1])
                    nc.vector.reciprocal_approx_fast(rcol[:, g_sl], den)
                    rc4 = rcol[:, g_sl]
                    rcol_bc = bass.AP(
                        tensor=rcol.tensor, offset=rc4.offset,
                        ap=[rcol.ap[0], [1, 4], [0, D]],
                    )
                    nc.vector.tensor_tensor(
                        out=y[:, g_sl, :], in0=ptr[:, :, 0:D], in1=rcol_bc,
                        op=ALU.mult,
                    )
                    nc.vector.tensor_tensor(
                        out=mus[:, g_sl], in0=ptr[:, :, D], in1=rc4, op=ALU.mult
                    )
                    nc.vector.tensor_add(
                        y[:, g_sl, :], y[:, g_sl, :], xr[:, g_sl, :]
                    )

                    # LN stats: mean from the free sum_d num column, variance
                    # from one squared reduce
                    ysq = yp.tile([128, 4, D], DT.bfloat16, tag="ysq")
                    nc.vector.tensor_mul(ysq, y[:, g_sl, :], y[:, g_sl, :])
                    nc.vector.reduce_sum(
                        ys2[:, g_sl], ysq, axis=mybir.AxisListType.X
                    )
                    # grp0 stats on Pool, grp1 on DVE: the two groups' serial
                    # LN chains then run in parallel (matters for the tail)
                    st_eng = nc.gpsimd if grp == 0 else nc.vector
                    st_eng.tensor_scalar(
                        mu[:, g_sl], mus[:, g_sl],
                        scalar1=1.0 / D, scalar2=None, op0=ALU.mult,
                    )
                    st_eng.tensor_add(mu[:, g_sl], mu[:, g_sl], xs[:, g_sl])
                    # var + eps = ys2/64 - mu^2 + 1e-5
                    st_eng.tensor_mul(m2[:, g_sl], mu[:, g_sl], mu[:, g_sl])
                    st_eng.tensor_scalar(
                        ve[:, g_sl], ys2[:, g_sl],
                        scalar1=1.0 / D, scalar2=1e-5, op0=ALU.mult, op1=ALU.add,
                    )
                    st_eng.tensor_sub(ve[:, g_sl], ve[:, g_sl], m2[:, g_sl])
                    nc.scalar.activation(sd[:, g_sl], ve[:, g_sl], AF.Sqrt)
                    nc.vector.reciprocal_approx_fast(rstd[:, g_sl], sd[:, g_sl])
                    nc.vector.tensor_copy(mub[:, g_sl], mu[:, g_sl])
                    nc.vector.tensor_copy(rstdb[:, g_sl], rstd[:, g_sl])

                    # normalize: out = (y - mu) * rstd
                    mub_bc = bass.AP(
                        tensor=mub.tensor, offset=mub[:, g_sl].offset,
                        ap=[mub.ap[0], [1, 4], [0, D]],
                    )
                    rstdb_bc = bass.AP(
                        tensor=rstdb.tensor, offset=rstdb[:, g_sl].offset,
                        ap=[rstdb.ap[0], [1, 4], [0, D]],
                    )
                    nc.vector.tensor_tensor(
                        out=out_sb[:, g_sl, :], in0=y[:, g_sl, :], in1=mub_bc,
                        op=ALU.subtract,
                    )
                    nc.vector.tensor_tensor(
                        out=out_sb[:, g_sl, :], in0=out_sb[:, g_sl, :],
                        in1=rstdb_bc, op=ALU.mult,
                    )
                    if apply_gamma_beta:
                        for t in range(grp * 4, grp * 4 + 4):
                            nc.gpsimd.tensor_mul(
                                out_sb[:, t, :], out_sb[:, t, :], gam_sb
                            )
                            nc.gpsimd.tensor_add(
                                out_sb[:, t, :], out_sb[:, t, :], bet_sb
                            )
                    nc.sync.dma_start(
                        out=out_dram[:, g_sl, :], in_=out_sb[:, g_sl, :]
                    )

            def make_stats():
                rcol = statp.tile([128, NT], DT.float32, tag="rcol")
                mus = statp.tile([128, NT], DT.float32, tag="mus")
                y = yp.tile([128, NT, D], DT.bfloat16, tag="y")
                ys2 = statp.tile([128, NT], DT.float32, tag="ys2")
                mu = statp.tile([128, NT], DT.float32, tag="mu")
                m2 = statp.tile([128, NT], DT.float32, tag="m2")
                ve = statp.tile([128, NT], DT.float32, tag="ve")
                sd = statp.tile([128, NT], DT.float32, tag="sd")
                rstd = statp.tile([128, NT], DT.float32, tag="rstd")
                mub = statp.tile([128, NT], DT.bfloat16, tag="mub")
                rstdb = statp.tile([128, NT], DT.bfloat16, tag="rstdb")
                out_sb = outp.tile([128, NT, D], DT.bfloat16, tag="o")
                return (rcol, mus, y, ys2, mu, m2, ve, sd, rstd, mub, rstdb,
                        out_sb)

            pending = None
            for b in range(BPC):
                wt, gt, v_sb, xr, xs = state[b]

                pot = psB.tile([128, N], DT.float32, tag="pot", name="pot")
                ebs = {}

                # ---- per j-tile: R via both PE row groups on disjoint column
                # halves of the same tile (adjacent emission -> they overlap),
                # then square, schraudolph-exp, PV accumulate (lagged one tile
                # so the PV matmuls never wait on the exp) ----
                for jt in range(NT):
                    pr = psA.tile([128, N], DT.float32, tag="pr", name="pr")
                    nc.tensor.matmul(
                        pr[:, 0:512],
                        lhsT=wt[0:D, jt * 128 : (jt + 1) * 128],
                        rhs=gt[0:D, 0:512],
                        tile_position=(0, 0),
                    )
                    nc.tensor.matmul(
                        pr[:, 512:N],
                        lhsT=wt[D : 2 * D, jt * 128 : (jt + 1) * 128],
                        rhs=gt[D : 2 * D, 512:N],
                        tile_position=(64, 0),
                    )
                    asq = asqp.tile([128, N], DT.float16, tag="asq")
                    if jt < K_ACT_SQ:
                        nc.scalar.activation(asq, pr, AF.Square)
                    else:
                        rf = asqp.tile([128, N], DT.float16, tag="rf")
                        nc.vector.tensor_copy(rf, pr)
                        nc.vector.tensor_mul(asq, rf, rf)
                    e = ep.tile([128, N], DT.uint16, tag="e")
                    nc.vector.tensor_scalar(
                        e, asq, scalar1=SCHRAUD_B, scalar2=None, op0=ALU.add
                    )
                    ebs[jt] = e.bitcast(DT.bfloat16)
                    pjs = [jt - 1] if jt < NT - 1 else [jt - 1, jt]
                    for pj in pjs:
                        if pj < 0:
                            continue
                        for c in range(2):
                            nc.tensor.matmul(
                                pot[0 : D + 2, c * 512 : (c + 1) * 512],
                                lhsT=v_sb[:, pj, 0 : D + 2],
                                rhs=ebs[pj][:, c * 512 : (c + 1) * 512],
                                start=(pj == 0),
                                stop=(pj == NT - 1),
                            )
                    if jt == 1 and pending is not None:
                        # previous batch's OUT^T -> SBUF copy (half DVE, half
                        # ACT), deferred so this batch's squares lead the ACT
                        # queue
                        pb, ppot, pxr, pxs = pending
                        ot = otp.tile([128, N], DT.bfloat16, tag="ot")
                        nc.vector.tensor_copy(
                            ot[0 : D + 2, 0:768], ppot[0 : D + 2, 0:768]
                        )
                        nc.scalar.copy(
                            ot[0 : D + 2, 768:N], ppot[0 : D + 2, 768:N]
                        )
                        pending = (pb, ot, pxr, pxs)
                    if jt == 2 and b + 1 < BPC:
                        state[b + 1] = emit_loads(b + 1)
                    if jt == 3 and pending is not None:
                        # previous batch's transposes + LN, deferred so the PE
                        # has this batch's matmuls in flight while the OT copy
                        # completes; split per group to spread the PE burst
                        pending_st = make_stats()
                        emit_epilogue_grp(*pending, pending_st, 0)
                    if jt == 5 and pending is not None:
                        emit_epilogue_grp(*pending, pending_st, 1)
                        pending = None

                pending = (b, pot, xr, xs)

            pb, ppot, pxr, pxs = pending
            ot = otp.tile([128, N], DT.bfloat16, tag="ot")
            nc.vector.tensor_copy(ot[0 : D + 2, 0:512], ppot[0 : D + 2, 0:512])
            nc.scalar.copy(ot[0 : D + 2, 512:N], ppot[0 : D + 2, 512:N])
            st = make_stats()
            emit_epilogue_grp(pb, ot, pxr, pxs, st, 0)
            emit_epilogue_grp(pb, ot, pxr, pxs, st, 1)

    nc.compile()
    return nc


_NC_CACHE: dict = {}


def kernel(input1, V_w, V_b, phi, ln_gamma, ln_beta, _trace=False):
    input1 = np.ascontiguousarray(np.asarray(input1, dtype=np.float32))
    V_w = np.asarray(V_w, dtype=np.float32)
    V_b = np.asarray(V_b, dtype=np.float32)
    phi = np.asarray(phi, dtype=np.float32)
    ln_gamma = np.asarray(ln_gamma, dtype=np.float32)
    ln_beta = np.asarray(ln_beta, dtype=np.float32)

    apply_gb = not (np.all(ln_gamma == 1.0) and np.all(ln_beta == 0.0))

    if apply_gb not in _NC_CACHE:
        _NC_CACHE[apply_gb] = _build_nc(apply_gb)
    nc = _NC_CACHE[apply_gb]

    O = _build_orthogonal(phi)
    ident = np.eye(128, dtype=np.float32)
    gb = np.ascontiguousarray(np.stack([ln_gamma, ln_beta]).astype(np.float32))

    # host prep: W' = x * ||x||^-1/2 * 8^-1/4 * (128/ln2)^1/4 (transposed f16),
    # gt = (W' O)^T, xo = X^T, xr = x + V_b (bf16 residual w/ folded bias)
    ss = (input1.astype(np.float64) ** 2).sum(-1)
    s = (ss ** -0.25 * 8.0 ** -0.25 * SCHRAUD_C4 ** 0.25).astype(np.float32)
    w = input1 * s[..., None]
    wt_full = np.ascontiguousarray(w.transpose(0, 2, 1).astype(np.float16))
    gt_full = np.ascontiguousarray(
        (w @ O).transpose(0, 2, 1).astype(np.float16)
    )
    vfull = input1 @ V_w.T
    v_full = np.ascontiguousarray(
        np.concatenate(
            [
                vfull,
                vfull.sum(-1, keepdims=True),
                np.ones((B, N, 1), np.float32),
            ],
            axis=-1,
        ).astype(ml_dtypes.bfloat16)
    )
    xr_f32 = input1 + V_b
    xr_full = np.ascontiguousarray(xr_f32.astype(ml_dtypes.bfloat16))
    xs_full = np.ascontiguousarray((xr_f32.sum(-1) / D).astype(np.float32))

    in_maps = []
    for c in range(NCORES):
        sl = slice(c * BPC, (c + 1) * BPC)
        in_maps.append(
            {
                "wt": wt_full[sl],
                "gt": gt_full[sl],
                "v": v_full[sl],
                "xr": xr_full[sl],
                "xs": xs_full[sl],
                "ident": ident,
                "gb": gb,
            }
        )

    res = bass_utils.run_bass_kernel_spmd(
        nc, in_maps, core_ids=list(range(NCORES)), trace=_trace
    )
    out = np.concatenate(
        [res.results[c]["out"].astype(np.float32) for c in range(NCORES)], axis=0
    )
    if _trace:
        kernel._last_result = res
    return out
